# revision 4
# baseline (speedup 1.0000x reference)
"""Trainium2 Bass kernel for nn_BinarizeLayer (chain Viterbi binarization).

Algorithm (scaled formulation)
------------------------------
The reference is a 2-state Viterbi DP over an 8.4M-node chain.  With
d_i = (a0_i - a1_i)/(2*lam) (scaled score difference) the forward pass is

    d_i = e_i + clamp(d_{i-1}, -1/2, 1/2),    e_i = (2*p_i - 1)/(2*lam),

and backtracking is  label_{i-1} = (d_{i-1} + label_i > 1/2)  with labels
in {0,1}: this single comparison replaces the usual gt/ge bit pair
(label==1 needs d >= -1/2, label==0 needs d > 1/2; adding the label to d
before one fixed threshold realizes both).

Conjugating by prefix sums of ebar = -e:  s_k = sum_{j<=k} ebar_j,
w_k = d_k + s_k + 1/2 turns the clamp into scan-expressible forms:

    w_k = min(max(w_{k-1}, s_{k-1}), s_{k-1} + 1)      (tensor_tensor_scan)
    label_{k-1} = (w_k + label_k > s_k + 1)            (reversed scan)

tensor_tensor_scan only exists on the Vector engine (codegen rejects it
on Pool), so the three scans (sum, walk, backtrack) run there, chained
across column chunks via initial=prev[:, -1:] — 24.9K columns/core at
~1 col/cycle @0.96GHz.  The two elementwise passes (ebar, SP=SB+1) run
on the Activation engine, overlapped chunk-by-chunk.  Labels leave as
int8.

Sharding: the chain is split into 8 core slices x 128 partition rows of
8192 payload elements; each row gets a 64-element halo on both sides
(the clamp walk and the backtrack both forget their initial state well
within 64 steps, so warm-up reproduces the exact sequential fp32 state).
The global chain ends are padded with p=0.5 (e=0 exactly), making the
boundary conditions exact: scan init 0.5 == d=0 before the first node,
and the reversed-scan init 0.5 implements the (d>0) final-label rule.
"""

import numpy as np

import concourse.bass as bass
import concourse.mybir as mybir
from concourse import tile
from concourse import bass_utils

LAM = 0.75
N = 8388608
NCORES = 8
P = 128          # partitions
W = 64           # halo / warm-up width
D = 8192         # payload elements per partition row
R = D + 2 * W    # row length incl. halos
FCH = 1040       # forward chunk width (8 chunks cover R)
NF = R // FCH
BCH = 1032       # backward chunk width (8 chunks cover [W, R))
NB = (R - W) // BCH


def _build():
    f32 = mybir.dt.float32
    i8 = mybir.dt.int8
    Alu = mybir.AluOpType
    Copy = mybir.ActivationFunctionType.Copy

    nc = bass.Bass()
    x = nc.dram_tensor("x", [P, R], f32, kind="ExternalInput")
    y = nc.dram_tensor("y", [P, D], i8, kind="ExternalOutput")

    inv2l = 1.0 / (2.0 * LAM)

    with tile.TileContext(nc) as tc:
        with tc.tile_pool(name="big", bufs=1) as big:
            XT = big.tile([P, R], f32)        # input p, then ebar in place
            SB = big.tile([P, R + 1], f32)    # [j] = exclusive prefix sum
            SP = big.tile([P, R + 1], f32)    # SB + 1
            WT = big.tile([P, R], f32)        # walk values
            LB = big.tile([P, R], i8)         # labels ([W, R) valid)

            nc.gpsimd.memset(SB[:, 0:1], 0.0)
            nc.gpsimd.memset(SP[:, 0:1], 1.0)

            # forward: DMA + ebar(Act) + sum scan + SP(Act) + walk scan
            for c in range(NF):
                c0, c1 = c * FCH, (c + 1) * FCH
                nc.sync.dma_start(XT[:, c0:c1], x[:, c0:c1])
                nc.scalar.activation(XT[:, c0:c1], XT[:, c0:c1],
                                     Copy, bias=inv2l, scale=-2.0 * inv2l)
                nc.vector.tensor_tensor_scan(
                    SB[:, c0 + 1:c1 + 1], XT[:, c0:c1], XT[:, c0:c1],
                    0.0 if c0 == 0 else SB[:, c0:c0 + 1],
                    Alu.add, Alu.bypass)
                nc.scalar.activation(SP[:, c0 + 1:c1 + 1],
                                     SB[:, c0 + 1:c1 + 1], Copy, bias=1.0)
                nc.vector.tensor_tensor_scan(
                    WT[:, c0:c1], SB[:, c0:c1], SP[:, c0:c1],
                    0.5 if c0 == 0 else WT[:, c0 - 1:c0],
                    Alu.max, Alu.min)

            # backtrack: reversed chained scans, right to left over [W, R)
            for c in range(NB - 1, -1, -1):
                b0, b1 = W + c * BCH, W + (c + 1) * BCH
                nc.vector.tensor_tensor_scan(
                    LB[:, b0:b1][:, ::-1],
                    WT[:, b0:b1][:, ::-1],
                    SP[:, b0 + 1:b1 + 1][:, ::-1],
                    0.5 if b1 == R else LB[:, b1:b1 + 1],
                    Alu.add, Alu.is_gt)
                nc.sync.dma_start(y[:, b0 - W:min(b1, W + D) - W],
                                  LB[:, b0:min(b1, W + D)])
    return nc


def _legalize_waits(nc, limit=1):
    """Split instructions carrying more than `limit` sem-waits.

    This walrus build rejects instructions whose sync_info has more wait
    commands than the ISA encoding allows (Tile can accumulate several).
    Excess waits move onto NoOps prepended on the same engine, which
    preserves per-engine ordering semantics.
    """
    import concourse.mybir as mybir
    for fn in nc.m.functions:
        for blk in fn.blocks:
            insts = blk.instructions
            i = 0
            while i < len(insts):
                inst = insts[i]
                si = getattr(inst, "sync_info", None)
                if si is not None and si.on_wait and len(si.on_wait) > limit:
                    waits = list(si.on_wait)
                    inst.sync_info = mybir.SyncInfo(
                        on_wait=waits[-limit:], on_update=list(si.on_update))
                    pending = waits[:-limit]
                    for j in range(0, len(pending), limit):
                        nop = mybir.InstNoOp(
                            name=nc.get_next_instruction_name(),
                            sync_info=mybir.SyncInfo(
                                on_wait=pending[j:j + limit], on_update=[]),
                            bass_nofuse=True,
                            engine=inst.engine,
                        )
                        insts.insert(i, nop)
                        i += 1
                i += 1
    return nc


_nc_cache = None


def _get_nc():
    global _nc_cache
    if _nc_cache is None:
        _nc_cache = _legalize_waits(_build())
    return _nc_cache


def _run(inputs: np.ndarray, **run_kwargs):
    p = np.ascontiguousarray(inputs, dtype=np.float32)
    assert p.shape == (N,)
    pad = np.full(W, 0.5, np.float32)
    pp = np.concatenate([pad, p, pad])
    nrows = N // D
    X = np.lib.stride_tricks.as_strided(pp, (nrows, R), (D * 4, 4))
    in_maps = [{"x": np.ascontiguousarray(X[k * P:(k + 1) * P])}
               for k in range(NCORES)]
    res = bass_utils.run_bass_kernel_spmd(_get_nc(), in_maps,
                                          core_ids=list(range(NCORES)),
                                          **run_kwargs)
    lab = np.concatenate([np.asarray(res.results[k]["y"]).reshape(-1)
                          for k in range(NCORES)])
    return lab.astype(np.int32), res


def kernel(inputs: np.ndarray) -> np.ndarray:
    return _run(inputs)[0]
